# revision 12
# baseline (speedup 1.0000x reference)
"""Causal self-attention (GQA, RoPE) Trainium2 Bass kernel.

Full inputs in, full output out. Tensor-parallel over heads across 8
NeuronCores: core i computes q-heads 4i..4i+3 (kv head i) and a partial
output projection over its 256 attn-out features; the host sums the 8
partial outputs (the "all-reduce after output_proj" step).

v4 design notes:
- x is transposed on the HOST and shipped as bf16 [E, S]; the qkv
  projection consumes it directly (no on-device x transposes).
- Attention in transposed layout: sT[k,q] = kT.T @ qT and
  outT[d,q] = v.T @ exp(sT); no attention transposes. Softmax
  denominators come from a ones-column in v; normalization is a rank-1
  broadcast matmul + DVE multiply per (head, q-block).
- Per-head features host-permuted to [evens, odds] so RoPE is
  contiguous elementwise; v/wo stay unpermuted.
- All matmul operands bf16 (FWL); PSUM fp32; bf16 partial outputs
  summed in fp32 on the host.
- Single fused schedule: the qkv projection of s-tile 4qb+4+h is
  emitted as PE filler inside attention slot (qb, h), interleaved with
  the score/AV chunk stream, so TensorE stays dense while ScalarE
  (exp) is the phase-2 rate limiter. xT blocks prefetch one block
  ahead; cos/sin tables are fully SBUF-resident.
"""

import numpy as np

import concourse.bacc as bacc
import concourse.mybir as mybir
import concourse.tile as tile
from concourse.bass_utils import run_bass_kernel_spmd

S = 2048          # sequence length
E = 2048          # embedding dim
H = 32            # query heads
KV = 8            # kv heads
HD = 64           # head dim
NCORES = 8
HC = H // NCORES  # query heads per core = 4
DQ = HC * HD      # per-core q proj width = 256
DKV = HD          # per-core kv proj width = 64
DQK = DQ + DKV    # roped span = 320
DW = DQ + 2 * DKV  # fused qkv proj width = 384
ST = S // 128     # 16 s-tiles of 128 rows
MASK_NEG = -1.0e4  # pre-scale additive mask (scaled: -1250 -> exp == 0)

F32 = mybir.dt.float32
F32R = mybir.dt.float32r
BF16 = mybir.dt.bfloat16


def r(ap):
    """Bitcast an AP to float32r so the PE runs fast-mode fp32 matmuls."""
    return ap.bitcast(F32R)


def build_nc(seq_tiles=ST, reps=1, phases=(1, 2, 3)):
    """Build + compile the per-core Bass program (identical on all cores)."""
    st_n = seq_tiles
    s_n = st_n * 128
    nit = st_n  # (q-block, head) attention slots

    nc = bacc.Bacc("TRN2", target_bir_lowering=False, debug=False)
    xt_d = nc.dram_tensor("xt", [E, s_n], BF16, kind="ExternalInput")
    wt_d = nc.dram_tensor("wt", [E, DW], BF16, kind="ExternalInput")
    wot_d = nc.dram_tensor("wot", [DQ, E], BF16, kind="ExternalInput")
    cos_d = nc.dram_tensor("cosh", [s_n, DQK // 2], F32, kind="ExternalInput")
    sin_d = nc.dram_tensor("sinh", [s_n, DQK // 2], F32, kind="ExternalInput")
    mask_d = nc.dram_tensor("maskadd", [128, 128], F32, kind="ExternalInput")
    id_d = nc.dram_tensor("ident", [128, 128], BF16, kind="ExternalInput")
    out_d = nc.dram_tensor("out", [s_n, E], BF16, kind="ExternalOutput")

    with tile.TileContext(nc) as tc:
        for _rep in range(reps):
            with (
                tc.tile_pool(name="const", bufs=1) as constp,
                tc.tile_pool(name="store", bufs=1) as storep,
            ):
                ident = constp.tile([128, 128], BF16)
                nc.sync.dma_start(out=ident[:], in_=id_d.ap()[:, :])
                wT_sb = constp.tile([128, E // 128, DW], BF16)
                nc.sync.dma_start(
                    out=wT_sb[:], in_=wt_d.ap().rearrange("(c p) f -> p c f", p=128)
                )
                maskT_sb = constp.tile([128, 128], F32)
                nc.sync.dma_start(out=maskT_sb[:], in_=mask_d.ap()[:, :])
                # cos/sin fully resident: [p, t, {cos,sin}, 160]
                cs_all = constp.tile([128, st_n, 2, DQK // 2], F32)
                nc.sync.dma_start(
                    out=cs_all[:, :, 0, :],
                    in_=cos_d.ap().rearrange("(t p) g -> p t g", p=128),
                )
                nc.sync.dma_start(
                    out=cs_all[:, :, 1, :],
                    in_=sin_d.ap().rearrange("(t p) g -> p t g", p=128),
                )
                woT_sb = constp.tile([128, 2, E], BF16)
                nc.sync.dma_start(
                    out=woT_sb[:], in_=wot_d.ap().rearrange("(c p) e -> p c e", p=128)
                )
                ones_sb = constp.tile([128, 128], F32)
                nc.vector.memset(ones_sb[:], 1.0)

                # qT: head h cols [h, s]; kT: [s]; d on partitions 0:64.
                qT_sb = storep.tile([64, HC, s_n], BF16)
                kT_sb = storep.tile([64, s_n], BF16)
                # v in [s, d] per 128-chunk; _ev has a ones col at 64 (den row
                # 64 of AV psum), _od has ones col at 0 + v at 64:128.
                v_ev = storep.tile([128, st_n, 72], BF16)
                v_od = storep.tile([128, st_n, 128], BF16)
                nc.vector.memset(v_ev[:, :, 64:72], 1.0)
                nc.vector.memset(v_od[:, :, 0:1], 1.0)
                nc.vector.memset(v_od[:, :, 1:64], 0.0)
                # attn-out transposed: feature d = c*128 + p, col = s.
                aoT_sb = storep.tile([128, 2, s_n], BF16)

                with (
                    tc.tile_pool(name="p1_x", bufs=2) as p1x,
                    tc.tile_pool(name="p1_sb", bufs=2) as p1,
                    tc.tile_pool(name="p2_aT", bufs=2) as p2a,
                    tc.tile_pool(name="p2_rt", bufs=2) as p2r,
                    tc.tile_pool(name="p3_o", bufs=2) as p3o,
                    tc.tile_pool(name="ps_qkv", bufs=2, space="PSUM") as ps_qkv_p,
                    tc.tile_pool(name="ps_tr", bufs=1, space="PSUM") as ps_tr_p,
                    tc.tile_pool(name="ps_s", bufs=3, space="PSUM") as ps_s_p,
                    tc.tile_pool(name="ps_av", bufs=2, space="PSUM") as ps_av_p,
                ):
                    xt_r = xt_d.ap().rearrange("(c p) s -> p c s", p=128)
                    xT_blk = [None] * (st_n // 4)

                    def load_blk(b):
                        if b < st_n // 4 and xT_blk[b] is None:
                            xT_blk[b] = p1x.tile(
                                [128, E // 128, 512], BF16, tag="xT", name="xTb"
                            )
                            nc.sync.dma_start(
                                out=xT_blk[b][:],
                                in_=xt_r[:, :, b * 512:(b + 1) * 512],
                            )

                    # ---- phase-1 emission pieces (software-pipelined) ----
                    rope_pend = [None]  # (qk_sb, t) awaiting transpose
                    tr_seq = [0]

                    def qkv_closures(t):
                        """16 PE closures accumulating tile t's qkv psum; the
                        last returns the psum for rope."""
                        blk, ts = t // 4, t % 4
                        ps_qkv = ps_qkv_p.tile([128, DW], F32, tag="qkv",
                                               name="psqkv")
                        ops = []
                        for c in range(E // 128):
                            def mm(c=c, t=t):
                                nc.tensor.matmul(
                                    ps_qkv[:],
                                    xT_blk[blk][:, c, ts * 128:(ts + 1) * 128],
                                    wT_sb[:, c, :],
                                    start=(c == 0),
                                    stop=(c == E // 128 - 1),
                                )
                            ops.append(mm)
                        return ops, ps_qkv

                    def emit_rope(ps_qkv, t):
                        """RoPE + v copies for tile t (DVE); marks tile pending
                        for transposes."""
                        se = ps_qkv[:, 0:DQK].rearrange(
                            "p (g two d) -> p two g d", two=2, d=32
                        )
                        qk_sb = p1.tile([128, DQK], BF16, tag="qkro", name="qkro")
                        de = qk_sb[:].rearrange("p (g two d) -> p two g d",
                                                two=2, d=32)
                        c_ap = cs_all[:, t, 0, :].rearrange("p (g d) -> p g d", d=32)
                        s_ap = cs_all[:, t, 1, :].rearrange("p (g d) -> p g d", d=32)
                        t1 = p1.tile([128, DQK // 2], F32, tag="t1")
                        t2 = p1.tile([128, DQK // 2], F32, tag="t2")
                        nc.vector.tensor_mul(t1[:], se[:, 0, :, :], c_ap)
                        nc.vector.tensor_mul(t2[:], se[:, 1, :, :], s_ap)
                        nc.vector.tensor_sub(de[:, 0, :, :], t1[:], t2[:])
                        t3 = p1.tile([128, DQK // 2], F32, tag="t3")
                        t4 = p1.tile([128, DQK // 2], F32, tag="t4")
                        nc.vector.tensor_mul(t3[:], se[:, 1, :, :], c_ap)
                        nc.vector.tensor_mul(t4[:], se[:, 0, :, :], s_ap)
                        nc.vector.tensor_add(de[:, 1, :, :], t3[:], t4[:])
                        nc.vector.tensor_copy(v_ev[:, t, 0:64], ps_qkv[:, DQK:DW])
                        nc.vector.tensor_copy(v_od[:, t, 64:128], ps_qkv[:, DQK:DW])
                        rope_pend[0] = (qk_sb, t)

                    def drain_tr():
                        """PE transposes + DVE copies for the pending roped
                        tile -> qT/kT."""
                        if rope_pend[0] is None:
                            return
                        qk_sb, t = rope_pend[0]
                        rope_pend[0] = None
                        ps_tr = ps_tr_p.tile([64, 5, 128], BF16, tag="tr",
                                             name="pstr")
                        for g in range(5):
                            nc.tensor.matmul(
                                ps_tr[:, g, :],
                                qk_sb[:, g * 64:(g + 1) * 64],
                                ident[:],
                                is_transpose=True,
                                start=(g == 0),
                                stop=(g == 4),
                            )
                        nc.vector.tensor_copy(
                            qT_sb[:, :, t * 128:(t + 1) * 128], ps_tr[:, 0:4, :]
                        )
                        nc.vector.tensor_copy(
                            kT_sb[:, t * 128:(t + 1) * 128], ps_tr[:, 4, :]
                        )

                    def emit_phase3(qb):
                        for st in range(4 * qb, 4 * qb + 4):
                            o_sb = p3o.tile([128, E], BF16, tag="o", name="osb")
                            for eb in range(E // 512):
                                ps_o = ps_av_p.tile([128, 512], F32, tag="av",
                                                    name="pso")
                                for c2 in range(2):
                                    nc.tensor.matmul(
                                        ps_o[:],
                                        aoT_sb[:, c2, st * 128:(st + 1) * 128],
                                        woT_sb[:, c2, eb * 512:(eb + 1) * 512],
                                        start=(c2 == 0),
                                        stop=(c2 == 1),
                                    )
                                nc.vector.tensor_copy(
                                    o_sb[:, eb * 512:(eb + 1) * 512], ps_o[:]
                                )
                            nc.gpsimd.dma_start(
                                out=out_d.ap()[st * 128:(st + 1) * 128, :],
                                in_=o_sb[:],
                            )

                    # ---- preamble: xT prefetch + s-tiles 0..3 ----
                    load_blk(0)
                    load_blk(1)
                    for t in range(4):
                        ops, ps_qkv = qkv_closures(t)
                        for op in ops:
                            op()
                        drain_tr()
                        emit_rope(ps_qkv, t)
                    drain_tr()

                    # ---- fused attention slots ----
                    aT_t = {}
                    for it in range(nit + 1 if 2 in phases else 0):
                        cur = it if it < nit else None
                        prv = it - 1 if it > 0 else None
                        filler = []
                        ps_qkv_cur = None
                        t_cur = None
                        if cur is not None:
                            qb, h = divmod(cur, 4)
                            k_cur = 4 * qb + 4
                            if h == 0:
                                load_blk(qb + 2)
                            drain_tr()
                            t_cur = 4 * qb + 4 + h
                            if t_cur < st_n:
                                filler, ps_qkv_cur = qkv_closures(t_cur)
                            aT = p2a.tile([128, st_n, 512], BF16, tag="aT",
                                          name="aT")
                            aT_t[cur] = aT
                        if prv is not None:
                            pq, ph = divmod(prv, 4)
                            k_prv = 4 * pq + 4
                            ps_av = ps_av_p.tile([128, 512], F32, tag="av",
                                                 name="psav")
                            pT = aT_t[prv]
                        nk = max(k_cur if cur is not None else 0,
                                 k_prv if prv is not None else 0)
                        fill_i = 0
                        for kc in range(nk):
                            if cur is not None and kc < k_cur:
                                lo = 128 * max(0, kc - 4 * qb)
                                ps_s = ps_s_p.tile([128, 512], F32, tag="s",
                                                   name="pss")
                                nc.tensor.matmul(
                                    ps_s[:, lo:512],
                                    kT_sb[:, kc * 128:(kc + 1) * 128],
                                    qT_sb[:, h, qb * 512 + lo:(qb + 1) * 512],
                                    start=True,
                                    stop=True,
                                )
                                if kc >= 4 * qb:  # diagonal chunk: causal mask
                                    nc.vector.tensor_add(
                                        ps_s[:, lo:lo + 128],
                                        ps_s[:, lo:lo + 128],
                                        maskT_sb[:],
                                    )
                                nc.scalar.activation(
                                    aT[:, kc, lo:512],
                                    ps_s[:, lo:512],
                                    mybir.ActivationFunctionType.Exp,
                                    scale=0.125,
                                )
                            if prv is not None and kc < k_prv:
                                lo = 128 * max(0, kc - 4 * pq)
                                vt = (v_ev[:, kc, 0:65] if ph % 2 == 0
                                      else v_od[:, kc, :])
                                npo = 65 if ph % 2 == 0 else 128
                                nc.tensor.matmul(
                                    ps_av[0:npo, lo:512],
                                    vt,
                                    pT[:, kc, lo:512],
                                    start=(kc == 0),
                                    stop=(kc == k_prv - 1),
                                )
                            for _ in range(2):
                                if fill_i < len(filler):
                                    filler[fill_i]()
                                    fill_i += 1
                        while fill_i < len(filler):
                            filler[fill_i]()
                            fill_i += 1
                        if ps_qkv_cur is not None:
                            emit_rope(ps_qkv_cur, t_cur)
                        if prv is not None:
                            # normalize: aoT = v-out rows * (1/den) bcast.
                            # approx-recip needs SBUF input at base partition
                            # 0: bounce den via ACT, recip rows [0:dr+1].
                            dr = 64 if ph % 2 == 0 else 0
                            den_sb = p2r.tile([128, 512], F32, tag="dn")
                            nc.scalar.copy(den_sb[dr:dr + 1, :],
                                           ps_av[dr:dr + 1, :])
                            rtmp = p2r.tile([128, 512], F32, tag="rt")
                            nc.vector.reciprocal_approx_fast(
                                rtmp[0:dr + 1, :], den_sb[0:dr + 1, :]
                            )
                            rinv = p2r.tile([128, 512], F32R, tag="ri")
                            nc.scalar.copy(rinv[dr:dr + 1, :], rtmp[dr:dr + 1, :])
                            ps_bc = ps_s_p.tile([128, 512], F32, tag="s",
                                                name="psbc")
                            nc.tensor.matmul(
                                ps_bc[:],
                                r(ones_sb[dr:dr + 1, :]),
                                rinv[dr:dr + 1, :],
                                start=True,
                                stop=True,
                            )
                            p0 = 64 * (ph & 1)
                            hp = ph >> 1
                            dst = aoT_sb[p0:p0 + 64, hp, pq * 512:(pq + 1) * 512]
                            nc.vector.tensor_copy(dst, ps_av[p0:p0 + 64, :])
                            nc.vector.tensor_mul(dst, dst, ps_bc[p0:p0 + 64, :])
                            if ph == 3 and 3 in phases:
                                emit_phase3(pq)

    nc.compile()
    return nc


def make_tables(s_n=S):
    """Host-side RoPE tables (pair-permuted layout) and causal maskT."""
    theta = (1.0 / (10000.0 ** (np.arange(0, HD, 2, dtype=np.float32) / HD))).astype(
        np.float32
    )
    freqs = np.arange(s_n, dtype=np.float32)[:, None] * theta[None, :]  # [s, 32]
    cos = np.cos(freqs).astype(np.float32)
    sin = np.sin(freqs).astype(np.float32)
    cosh = np.tile(cos, (1, DQK // HD))  # [s, 160] (5 groups of 32)
    sinh = np.tile(sin, (1, DQK // HD))
    a = np.arange(128)
    # sT layout: rows = k, cols = q; mask out k > q.
    maskadd = np.where(a[:, None] <= a[None, :], 0.0, MASK_NEG).astype(np.float32)
    return cosh, sinh, maskadd


def _bf16(x):
    import ml_dtypes
    return np.ascontiguousarray(x).astype(ml_dtypes.bfloat16)


# per-head feature permutation: evens then odds
_PERM = np.concatenate([np.arange(0, HD, 2), np.arange(1, HD, 2)])


def make_core_inputs(x2, wq, wk, wv, wo, core):
    """Per-core input dict (host-side sharding prep)."""
    cosh, sinh, maskadd = _TABLES
    i = core
    wq_i = wq[i * DQ:(i + 1) * DQ].reshape(HC, HD, E)[:, _PERM, :].reshape(DQ, E)
    wk_i = wk[i * DKV:(i + 1) * DKV][_PERM, :]
    wv_i = wv[i * DKV:(i + 1) * DKV]
    wt = np.concatenate([wq_i, wk_i, wv_i], axis=0).T
    wot = wo[:, i * DQ:(i + 1) * DQ].T
    return {
        "xt": _bf16(x2.T),
        "wt": _bf16(wt),
        "wot": _bf16(wot),
        "cosh": cosh,
        "sinh": sinh,
        "maskadd": maskadd,
        "ident": _bf16(np.eye(128, dtype=np.float32)),
    }


_TABLES = make_tables()
_NC_CACHE = {}


def _get_nc(reps=1):
    key = ("nc", reps)
    if key not in _NC_CACHE:
        _NC_CACHE[key] = build_nc(reps=reps)
    return _NC_CACHE[key]


def kernel(x, wq, wk, wv, wo):
    x = np.asarray(x, dtype=np.float32)
    b, s_n, e = x.shape
    x2 = np.ascontiguousarray(x.reshape(s_n, e))
    in_maps = [
        make_core_inputs(x2, np.asarray(wq, np.float32), np.asarray(wk, np.float32),
                         np.asarray(wv, np.float32), np.asarray(wo, np.float32), i)
        for i in range(NCORES)
    ]
    res = run_bass_kernel_spmd(_get_nc(), in_maps, core_ids=list(range(NCORES)))
    out = np.zeros((s_n, e), dtype=np.float32)
    for rr in res.results:
        out += np.asarray(rr["out"]).astype(np.float32)
    return out.reshape(b, s_n, e).astype(np.float32)
